# revision 6
# baseline (speedup 1.0000x reference)
"""Trainium2 Bass kernel for AttentionBlock (B=4, C=256, H=W=64).

Sharding: 8 cores = (batch b, query-half h). Each core holds the full
x[b] (for K/V over all 4096 positions) and computes attention output for
its 2048 query positions. Host permutes x columns so the core's own
query half is first (order of j is irrelevant: softmax+PV contract over
all j).

Per-core dataflow (all on one NeuronCore, Tile framework):
  q = WqT.T @ x[:, :2048] + bq          [32, 2048]
  k = WkT.T @ x + bk                    [32, 4096]
  vT[j, c] = x.T @ WvT                  [4096, 256] (32 chunks of [128, 256])
  for each i-superblock (512 queries):
    for each j-chunk (128 keys):
      eT[j, i] = k_chunk.T @ q_blk      (PSUM, f32)   -- energy, transposed
      ex = exp(eT)                      (ACT, PSUM->SBUF)
      pv[c, i]   += vT_chunk.T @ ex     (PSUM accumulate over j)
      sums[1, i] += ones.T @ ex         (PSUM accumulate over j)
    rg = gamma / sums                   (DVE reciprocal + ACT scale)
    bc[128, i] = ones_col.T @ rg        (K=1 matmul broadcast)
    out[c, i] = pv * bc + (gamma*bv + x[:, i])   (DVE)
Notes:
 - softmax rows sum to 1, so the v-bias contributes exactly gamma*bv[c]
   to the output; vT is computed bias-free and bv is folded into the
   final elementwise op.
 - softmax is computed without max subtraction: energies are in
   [-45, 42] for this distribution, well within f32 exp range.
 - matmul operands use float32r (full-rate fp32 matmul on TRN2).
"""

import numpy as np

import concourse.bass as bass
import concourse.mybir as mybir
import concourse.tile as tile
from concourse import bacc
from concourse.bass_utils import run_bass_kernel_spmd

AF = mybir.ActivationFunctionType
OP = mybir.AluOpType
F32 = mybir.dt.float32
F32R = mybir.dt.float32r

B, C, HH, WW = 4, 256, 64, 64
N = HH * WW          # 4096 spatial positions
CQ = 32              # q/k channels
NCORES = 8
NQ = N // 2          # 2048 queries per core
P = 128
FB = 512             # free-dim block (one PSUM bank of f32)
JCH = N // P         # 32 j-chunks
ISB = NQ // FB       # 4 i-superblocks
NCH = C // P         # 2 channel chunks
GRP = 2              # j-chunks per energy/exp group


def _emit_body(nc, tc, d):
    """Emit one full forward pass. d: dict of DRAM APs."""
    with (
        tc.tile_pool(name="const", bufs=1) as cpool,
        tc.tile_pool(name="xp", bufs=1) as xpool,
        tc.tile_pool(name="kq", bufs=1) as kqpool,
        tc.tile_pool(name="vt", bufs=1) as vtpool,
    ):
        # ---- constant / weight loads (per 128-partition chunk) ----
        wq_sb, wk_sb, wv_sb, bv_sb = [], [], [], []
        for cc in range(NCH):
            csl = bass.ts(cc, P)
            t = cpool.tile([P, CQ], F32R, tag=f"wq{cc}", name=f"wq{cc}")
            nc.sync.dma_start(t[:], d["wqT"][csl, :])
            wq_sb.append(t)
            t = cpool.tile([P, CQ], F32R, tag=f"wk{cc}", name=f"wk{cc}")
            nc.sync.dma_start(t[:], d["wkT"][csl, :])
            wk_sb.append(t)
            t = cpool.tile([P, C], F32R, tag=f"wv{cc}", name=f"wv{cc}")
            nc.sync.dma_start(t[:], d["wvT"][csl, :])
            wv_sb.append(t)
            t = cpool.tile([P, 1], F32, tag=f"bvg{cc}", name=f"bvg{cc}")
            nc.sync.dma_start(t[:], d["bvg"][csl, :])
            bv_sb.append(t)
        bq_sb = cpool.tile([CQ, 1], F32, tag="bq")
        nc.sync.dma_start(bq_sb[:], d["bq"][:])
        bk_sb = cpool.tile([CQ, 1], F32, tag="bk")
        nc.sync.dma_start(bk_sb[:], d["bk"][:])
        gam_sb = cpool.tile([1, 1], F32, tag="gam")
        nc.sync.dma_start(gam_sb[:], d["gam"][:])
        ones_sb = cpool.tile([P, 1], F32R, tag="ones")
        nc.sync.dma_start(ones_sb[:], d["ones"][:])
        onesm_sb = cpool.tile([1, P], F32R, tag="onesm")
        nc.sync.dma_start(onesm_sb[:], d["onesm"][:])

        # ---- x: [256, 4096] as 2 partition-chunks, DMA'd in column blocks ----
        XBLK = 1024
        x_sb = []
        for cc in range(NCH):
            t = xpool.tile([P, N], F32R, tag=f"x{cc}", name=f"x{cc}")
            x_sb.append(t)
        for blk in range(N // XBLK):
            sl = bass.ts(blk, XBLK)
            for cc in range(NCH):
                nc.sync.dma_start(x_sb[cc][:, sl], d["x"][cc * P:(cc + 1) * P, sl])

        # ---- projections ----
        with tc.tile_pool(name="ps_proj", bufs=4, space="PSUM") as psproj:
            q_sb = kqpool.tile([CQ, NQ], F32R, tag="q")
            k_sb = kqpool.tile([CQ, N], F32R, tag="k")
            vt_sb = [vtpool.tile([P, C], F32R, tag=f"vt{j}", name=f"vt{j}")
                     for j in range(JCH)]

            # q: [32, 2048] over x[:, :2048]
            for nb in range(NQ // FB):
                ps = psproj.tile([P, FB], F32, tag="psp", name="psq")[0:CQ, :]
                for cc in range(NCH):
                    nc.tensor.matmul(
                        ps[:], wq_sb[cc][:], x_sb[cc][:, bass.ts(nb, FB)],
                        start=(cc == 0), stop=(cc == NCH - 1),
                    )
                nc.scalar.activation(q_sb[:, bass.ts(nb, FB)], ps[:], AF.Identity,
                                     bias=bq_sb[:, 0:1])

            # k: [32, 4096]
            for nb in range(N // FB):
                ps = psproj.tile([P, FB], F32, tag="psp", name="psk")[0:CQ, :]
                for cc in range(NCH):
                    nc.tensor.matmul(
                        ps[:], wk_sb[cc][:], x_sb[cc][:, bass.ts(nb, FB)],
                        start=(cc == 0), stop=(cc == NCH - 1),
                    )
                nc.scalar.activation(k_sb[:, bass.ts(nb, FB)], ps[:], AF.Identity,
                                     bias=bk_sb[:, 0:1])

            # vT: 32 chunks of [128 j, 256 c] = x_chunk.T @ WvT (bias folded out)
            for j in range(JCH):
                ps = psproj.tile([P, FB], F32, tag="psp", name="psv")[:, 0:C]
                for cc in range(NCH):
                    nc.tensor.matmul(
                        ps[:], x_sb[cc][:, bass.ts(j, P)], wv_sb[cc][:],
                        start=(cc == 0), stop=(cc == NCH - 1),
                    )
                nc.scalar.copy(vt_sb[j][:], ps[:])

        # ---- attention over i-superblocks ----
        with (
            tc.tile_pool(name="ex", bufs=4) as expool,
            tc.tile_pool(name="fin", bufs=4) as fpool,
            tc.tile_pool(name="ps_e", bufs=2, space="PSUM") as pse,
            tc.tile_pool(name="ps_acc", bufs=1, space="PSUM") as psacc,
        ):
            for isb in range(ISB):
                isl = bass.ts(isb, FB)
                pv = [psacc.tile([P, FB], F32, tag=f"pv{cc}", name=f"pv{cc}")
                      for cc in range(NCH)]
                sm = psacc.tile([1, FB], F32, tag="sm")

                # software pipeline: energy/exp one group ahead of PV
                pending = None
                for g in range(JCH // GRP):
                    pe_t = pse.tile([P, GRP * FB], F32, tag="pe")
                    for jj in range(GRP):
                        j = GRP * g + jj
                        nc.tensor.matmul(
                            pe_t[:, bass.ts(jj, FB)],
                            k_sb[:, bass.ts(j, P)],
                            q_sb[:, isl],
                            start=True, stop=True,
                        )
                    ex_t = expool.tile([P, GRP * FB], F32R, tag="ex")
                    nc.scalar.activation(ex_t[:], pe_t[:], AF.Exp)
                    if pending is not None:
                        _emit_pv(nc, pending[0], pending[1], pv, sm, vt_sb, ones_sb)
                    pending = (g, ex_t)
                _emit_pv(nc, pending[0], pending[1], pv, sm, vt_sb, ones_sb)

                # normalization: rg = gamma / sums, broadcast to 128 partitions
                recip_sb = fpool.tile([1, FB], F32, tag="recip")
                nc.vector.reciprocal(recip_sb[:], sm[:])
                rg_sb = fpool.tile([1, FB], F32R, tag="rg")
                nc.scalar.activation(rg_sb[:], recip_sb[:], AF.Copy,
                                     scale=gam_sb[0:1, 0:1])
                bc_ps = psacc.tile([P, FB], F32, tag="bc")
                nc.tensor.matmul(bc_ps[:], onesm_sb[:], rg_sb[:],
                                 start=True, stop=True)
                bc_sb = fpool.tile([P, FB], F32, tag="bc_sb")
                nc.scalar.copy(bc_sb[:], bc_ps[:])

                # out = pv * bc + (gamma*bv + x_residual)
                for cc in range(NCH):
                    tmp = fpool.tile([P, FB], F32, tag="tmp")
                    nc.vector.tensor_tensor(tmp[:], pv[cc][:], bc_sb[:], op=OP.mult)
                    o_sb = fpool.tile([P, FB], F32, tag="osb")
                    nc.vector.scalar_tensor_tensor(
                        o_sb[:], tmp[:], bv_sb[cc][:, 0:1],
                        x_sb[cc][:, isl].bitcast(F32),
                        op0=OP.add, op1=OP.add,
                    )
                    nc.sync.dma_start(d["out"][cc * P:(cc + 1) * P, isl], o_sb[:])


def _emit_pv(nc, g, ex_t, pv, sm, vt_sb, ones_sb):
    for jj in range(GRP):
        j = GRP * g + jj
        exsl = ex_t[:, bass.ts(jj, FB)]
        for cc in range(NCH):
            nc.tensor.matmul(
                pv[cc][:],
                vt_sb[j][:, cc * P:(cc + 1) * P],
                exsl,
                start=(j == 0), stop=(j == JCH - 1),
            )
        nc.tensor.matmul(
            sm[:],
            ones_sb[:, 0:1],
            exsl,
            start=(j == 0), stop=(j == JCH - 1),
        )


_programs = {}


def build_program(repeat=1):
    if repeat in _programs:
        return _programs[repeat]
    nc = bacc.Bacc("TRN2", target_bir_lowering=False, debug=False,
                   num_devices=NCORES)
    d = {
        "x": nc.dram_tensor("x", [C, N], F32R, kind="ExternalInput").ap(),
        "wqT": nc.dram_tensor("wqT", [C, CQ], F32R, kind="ExternalInput").ap(),
        "wkT": nc.dram_tensor("wkT", [C, CQ], F32R, kind="ExternalInput").ap(),
        "wvT": nc.dram_tensor("wvT", [C, C], F32R, kind="ExternalInput").ap(),
        "bq": nc.dram_tensor("bq", [CQ, 1], F32, kind="ExternalInput").ap(),
        "bk": nc.dram_tensor("bk", [CQ, 1], F32, kind="ExternalInput").ap(),
        "bvg": nc.dram_tensor("bvg", [C, 1], F32, kind="ExternalInput").ap(),
        "gam": nc.dram_tensor("gam", [1, 1], F32, kind="ExternalInput").ap(),
        "ones": nc.dram_tensor("ones", [P, 1], F32R, kind="ExternalInput").ap(),
        "onesm": nc.dram_tensor("onesm", [1, P], F32R, kind="ExternalInput").ap(),
        "out": nc.dram_tensor("out", [C, NQ], F32, kind="ExternalOutput").ap(),
    }
    with tile.TileContext(nc) as tc:
        for _ in range(repeat):
            _emit_body(nc, tc, d)
    nc.compile()
    _programs[repeat] = nc
    return nc


def make_in_maps(x, Wq, bq, Wk, bk, Wv, bv, gamma):
    x = np.asarray(x, dtype=np.float32)
    Wq = np.asarray(Wq, dtype=np.float32)
    bq = np.asarray(bq, dtype=np.float32)
    Wk = np.asarray(Wk, dtype=np.float32)
    bk = np.asarray(bk, dtype=np.float32)
    Wv = np.asarray(Wv, dtype=np.float32)
    bv = np.asarray(bv, dtype=np.float32)
    gamma = np.asarray(gamma, dtype=np.float32)

    shared = {
        "wqT": np.ascontiguousarray(Wq.T),
        "wkT": np.ascontiguousarray(Wk.T),
        "wvT": np.ascontiguousarray(Wv.T),
        "bq": np.ascontiguousarray(bq[:, None]),
        "bk": np.ascontiguousarray(bk[:, None]),
        # softmax rows sum to 1 => v-bias contributes gamma*bv to output
        "bvg": np.ascontiguousarray((gamma.reshape(()) * bv)[:, None]),
        "gam": gamma.reshape(1, 1),
        "ones": np.ones((P, 1), np.float32),
        "onesm": np.ones((1, P), np.float32),
    }
    in_maps = []
    for core in range(NCORES):
        b, h = core // 2, core % 2
        xb = x[b].reshape(C, N)
        xr = np.concatenate(
            [xb[:, h * NQ:(h + 1) * NQ], xb[:, (1 - h) * NQ:(2 - h) * NQ]],
            axis=1)
        m = dict(shared)
        m["x"] = np.ascontiguousarray(xr)
        in_maps.append(m)
    return in_maps


def assemble_output(results, dtype=np.float32):
    out = np.empty((B, C, N), np.float32)
    for core in range(NCORES):
        b, h = core // 2, core % 2
        out[b][:, h * NQ:(h + 1) * NQ] = results[core]["out"]
    return out.reshape(B, C, HH, WW).astype(dtype, copy=False)


def kernel(x, Wq, bq, Wk, bk, Wv, bv, gamma):
    nc = build_program(repeat=1)
    in_maps = make_in_maps(x, Wq, bq, Wk, bk, Wv, bv, gamma)
    res = run_bass_kernel_spmd(nc, in_maps, list(range(NCORES)))
    return assemble_output(res.results, dtype=np.asarray(x).dtype)


# revision 10
# speedup vs baseline: 464.8551x; 464.8551x over previous
"""Trainium2 Bass kernel for AttentionBlock (B=4, C=256, H=W=64).

Sharding: 8 cores = (batch b, query-half h). Each core holds the full
x[b] (for K over all 4096 positions) and computes attention output for
its 2048 query positions. Host permutes x columns so the core's own
query half is first (order of j is irrelevant: softmax+PV contract over
all j). Host also supplies xT (x transposed) so the value contraction
needs no on-chip transposes.

Per-core dataflow (Tile framework, one NeuronCore):
  q = WqT.T @ x[:, :2048] + bq           [32, 2048]
  k = WkT.T @ x + bk                     [32, 4096]
  for each i-superblock (512 queries):
    for each j-chunk (128 keys):
      eT[j, i] = k_chunk.T @ q_blk       (PSUM f32; no max subtraction --
                                          energies are in [-45, 42])
      ex = exp(eT)                       (ACT, PSUM->SBUF, f32r)
      z[cin, i]  += xT_chunk.T @ ex      (PSUM; reassociated value path:
                                          out = Wv (x attn) since v = Wv x)
      sums[1, i] += ones.T @ ex          (PSUM)
    zs = copy(z)                         (ACT, f32r)
    out_ps[cout, i] = WvT_chunk.T @ zs   (PSUM)
    rg = gamma / sums                    (DVE reciprocal + ACT scale)
    bc[128, i] = ones_col.T @ rg         (K=1 matmul broadcast)
    out = out_ps * bc + (gamma*bv + x[:, i])  (DVE; softmax rows sum to 1
                                          so v-bias folds to +gamma*bv)
All matmul operands are float32r (full-rate fp32 matmul on TRN2).
"""

import numpy as np

import concourse.bass as bass
import concourse.mybir as mybir
import concourse.tile as tile
from concourse import bacc
from concourse.bass_utils import run_bass_kernel_spmd

AF = mybir.ActivationFunctionType
OP = mybir.AluOpType
F32 = mybir.dt.float32
F32R = mybir.dt.float32r

B, C, HH, WW = 4, 256, 64, 64
N = HH * WW          # 4096 spatial positions
CQ = 32              # q/k channels
NCORES = 8
NQ = N // 2          # 2048 queries per core
P = 128
FB = 512             # free-dim block (one PSUM bank of f32)
JCH = N // P         # 32 j-chunks
ISB = NQ // FB       # 4 i-superblocks
NCH = C // P         # 2 channel chunks
GRP = 4              # j-chunks per energy/exp group


def _emit_body(nc, tc, d):
    """Emit one full forward pass. d: dict of DRAM APs."""
    with (
        tc.tile_pool(name="const", bufs=1) as cpool,
        tc.tile_pool(name="xp", bufs=1) as xpool,
        tc.tile_pool(name="kq", bufs=1) as kqpool,
    ):
        # ---- x: [256, 4096] as 2 partition-chunks; first block DMA'd first
        #      so projections can start ASAP ----
        XBLK = 1024
        x_sb = []
        for cc in range(NCH):
            t = xpool.tile([P, N], F32R, tag=f"x{cc}", name=f"x{cc}")
            x_sb.append(t)
        for cc in range(NCH):
            nc.sync.dma_start(x_sb[cc][:, 0:XBLK], d["x"][cc * P:(cc + 1) * P, 0:XBLK])

        # ---- weights needed by q/k projections ----
        wq_sb, wk_sb, wv_sb, bv_sb = [], [], [], []
        for cc in range(NCH):
            csl = bass.ts(cc, P)
            t = cpool.tile([P, CQ], F32R, tag=f"wq{cc}", name=f"wq{cc}")
            nc.sync.dma_start(t[:], d["wqT"][csl, :])
            wq_sb.append(t)
            t = cpool.tile([P, CQ], F32R, tag=f"wk{cc}", name=f"wk{cc}")
            nc.sync.dma_start(t[:], d["wkT"][csl, :])
            wk_sb.append(t)
        bq_sb = cpool.tile([CQ, 1], F32, tag="bq")
        nc.sync.dma_start(bq_sb[:], d["bq"][:])
        bk_sb = cpool.tile([CQ, 1], F32, tag="bk")
        nc.sync.dma_start(bk_sb[:], d["bk"][:])

        # ---- remaining x blocks ----
        for blk in range(1, N // XBLK):
            sl = bass.ts(blk, XBLK)
            for cc in range(NCH):
                nc.sync.dma_start(x_sb[cc][:, sl], d["x"][cc * P:(cc + 1) * P, sl])

        # ---- remaining constants ----
        for cc in range(NCH):
            csl = bass.ts(cc, P)
            t = cpool.tile([P, C], F32R, tag=f"wv{cc}", name=f"wv{cc}")
            nc.sync.dma_start(t[:], d["wvT"][csl, :])
            wv_sb.append(t)
            t = cpool.tile([P, 1], F32, tag=f"bvg{cc}", name=f"bvg{cc}")
            nc.sync.dma_start(t[:], d["bvg"][csl, :])
            bv_sb.append(t)
        gam_sb = cpool.tile([1, 1], F32, tag="gam")
        nc.sync.dma_start(gam_sb[:], d["gam"][:])
        ones_sb = cpool.tile([P, 1], F32R, tag="ones")
        nc.sync.dma_start(ones_sb[:], d["ones"][:])

        # ---- xT: [4096, 256] -> one tile [128, 32*256]:
        #      partition p, free (a, c) with j = a*128 + p ----
        xt_sb = xpool.tile([P, JCH * C], F32R, tag="xt", name="xt")
        xt_view = d["xT"].rearrange("(a p) c -> p a c", p=P)   # [128, 32, 256]
        for ab in range(2):
            asl = bass.ts(ab, JCH // 2)
            nc.sync.dma_start(
                xt_sb[:, ab * (JCH // 2) * C:(ab + 1) * (JCH // 2) * C],
                xt_view[:, asl, :])

        # ---- q/k projections, interleaved by column block ----
        with tc.tile_pool(name="ps_proj", bufs=4, space="PSUM") as psproj:
            q_sb = kqpool.tile([CQ, NQ], F32R, tag="q")
            k_sb = kqpool.tile([CQ, N], F32R, tag="k")

            def proj(which, nb):
                w_sb, b_sb, o_sb = ((wq_sb, bq_sb, q_sb) if which == "q"
                                    else (wk_sb, bk_sb, k_sb))
                ps = psproj.tile([P, FB], F32, tag="psp", name="psp")[0:CQ, :]
                for cc in range(NCH):
                    nc.tensor.matmul(
                        ps[:], w_sb[cc][:], x_sb[cc][:, bass.ts(nb, FB)],
                        start=(cc == 0), stop=(cc == NCH - 1),
                    )
                nc.vector.tensor_scalar(o_sb[:, bass.ts(nb, FB)], ps[:],
                                        b_sb[:, 0:1], None, op0=OP.add)

            for blk in range(N // XBLK):
                for nb in range(blk * XBLK // FB, (blk + 1) * XBLK // FB):
                    if nb < NQ // FB:
                        proj("q", nb)
                    proj("k", nb)

        # ---- attention over i-superblocks (software-pipelined) ----
        with (
            tc.tile_pool(name="ex", bufs=3) as expool,
            tc.tile_pool(name="fin", bufs=4) as fpool,
            tc.tile_pool(name="ps_e", bufs=1, space="PSUM") as pse,
            tc.tile_pool(name="ps_acc", bufs=1, space="PSUM") as psacc,
        ):
            NG = JCH // GRP
            states = []

            def emit_eexp(state, g):
                pe_t = pse.tile([P, GRP * FB], F32, tag="pe", name="pe")
                for jj in range(GRP):
                    j = GRP * g + jj
                    nc.tensor.matmul(
                        pe_t[:, bass.ts(jj, FB)],
                        k_sb[:, bass.ts(j, P)],
                        q_sb[:, state["isl"]],
                        start=True, stop=True,
                    )
                ex_t = expool.tile([P, GRP * FB], F32R, tag="ex", name="ex")
                nc.scalar.activation(ex_t[:], pe_t[:], AF.Exp)
                state["exps"][g] = ex_t

            def emit_zg(state, g):
                if state["z"] is None:
                    state["z"] = [
                        psacc.tile([P, FB], F32, tag=f"z{cc}", name=f"z{cc}")
                        for cc in range(NCH)]
                    state["sm"] = psacc.tile([1, FB], F32, tag="sm", name="sm")
                ex_t = state["exps"].pop(g)
                for jj in range(GRP):
                    j = GRP * g + jj
                    exsl = ex_t[:, bass.ts(jj, FB)]
                    for cc in range(NCH):
                        nc.tensor.matmul(
                            state["z"][cc][:],
                            xt_sb[:, j * C + cc * P: j * C + (cc + 1) * P],
                            exsl,
                            start=(j == 0), stop=(j == JCH - 1),
                        )
                    nc.tensor.matmul(
                        state["sm"][:],
                        ones_sb[:, 0:1],
                        exsl,
                        start=(j == 0), stop=(j == JCH - 1),
                    )

            def emit_tail_a(state):
                # z -> SBUF (f32r) on DVE; normalizer on DVE + Pool
                state["zs"] = []
                for cc in range(NCH):
                    t = fpool.tile([P, FB], F32R, tag=f"zs{cc}", name=f"zs{cc}")
                    nc.vector.tensor_copy(t[:], state["z"][cc][:])
                    state["zs"].append(t)
                recip_sb = fpool.tile([1, FB], F32, tag="recip", name="recip")
                nc.vector.reciprocal(recip_sb[:], state["sm"][:])
                rg_sb = fpool.tile([1, FB], F32, tag="rg", name="rg")
                nc.vector.tensor_scalar(rg_sb[:], recip_sb[:],
                                        gam_sb[0:1, 0:1], None, op0=OP.mult)
                bc_sb = fpool.tile([P, FB], F32, tag="bc_sb", name="bc_sb")
                nc.gpsimd.partition_broadcast(bc_sb[:], rg_sb[0:1, :])
                state["bc"] = bc_sb

            def emit_tail_b(state):
                # out[cout] = (Wv z) * bc + (gamma*bv + x_residual)
                isl = state["isl"]
                for co in range(NCH):
                    ops = psacc.tile([P, FB], F32, tag="ops", name="ops")
                    for ci in range(NCH):
                        nc.tensor.matmul(
                            ops[:],
                            wv_sb[ci][:, co * P:(co + 1) * P],
                            state["zs"][ci][:],
                            start=(ci == 0), stop=(ci == NCH - 1),
                        )
                    tmp = fpool.tile([P, FB], F32, tag="tmp", name="tmp")
                    nc.vector.tensor_tensor(tmp[:], ops[:], state["bc"][:],
                                            op=OP.mult)
                    o_sb = fpool.tile([P, FB], F32, tag="osb", name="osb")
                    nc.vector.scalar_tensor_tensor(
                        o_sb[:], tmp[:], bv_sb[co][:, 0:1],
                        x_sb[co][:, isl].bitcast(F32),
                        op0=OP.add, op1=OP.add,
                    )
                    nc.sync.dma_start(d["out"][co * P:(co + 1) * P, isl], o_sb[:])

            for isb in range(ISB):
                state = {"isl": bass.ts(isb, FB), "z": None, "sm": None,
                         "exps": {}, "zs": None, "bc": None}
                states.append(state)
                for g in range(NG):
                    emit_eexp(state, g)
                    if isb >= 1:
                        prev = states[isb - 1]
                        if g == 0:
                            emit_zg(prev, NG - 1)
                            emit_tail_a(prev)
                        elif g == 1:
                            emit_tail_b(prev)
                    if g >= 1:
                        emit_zg(state, g - 1)
            last = states[-1]
            emit_zg(last, NG - 1)
            emit_tail_a(last)
            emit_tail_b(last)

_programs = {}


def build_program(repeat=1):
    if repeat in _programs:
        return _programs[repeat]
    nc = bacc.Bacc("TRN2", target_bir_lowering=False, debug=False,
                   num_devices=NCORES)
    d = {
        "x": nc.dram_tensor("x", [C, N], F32R, kind="ExternalInput").ap(),
        "xT": nc.dram_tensor("xT", [N, C], F32R, kind="ExternalInput").ap(),
        "wqT": nc.dram_tensor("wqT", [C, CQ], F32R, kind="ExternalInput").ap(),
        "wkT": nc.dram_tensor("wkT", [C, CQ], F32R, kind="ExternalInput").ap(),
        "wvT": nc.dram_tensor("wvT", [C, C], F32R, kind="ExternalInput").ap(),
        "bq": nc.dram_tensor("bq", [CQ, 1], F32, kind="ExternalInput").ap(),
        "bk": nc.dram_tensor("bk", [CQ, 1], F32, kind="ExternalInput").ap(),
        "bvg": nc.dram_tensor("bvg", [C, 1], F32, kind="ExternalInput").ap(),
        "gam": nc.dram_tensor("gam", [1, 1], F32, kind="ExternalInput").ap(),
        "ones": nc.dram_tensor("ones", [P, 1], F32R, kind="ExternalInput").ap(),
        "out": nc.dram_tensor("out", [C, NQ], F32, kind="ExternalOutput").ap(),
    }
    with tile.TileContext(nc) as tc:
        for _ in range(repeat):
            _emit_body(nc, tc, d)
    nc.compile()
    _programs[repeat] = nc
    return nc


def make_in_maps(x, Wq, bq, Wk, bk, Wv, bv, gamma):
    x = np.asarray(x, dtype=np.float32)
    Wq = np.asarray(Wq, dtype=np.float32)
    bq = np.asarray(bq, dtype=np.float32)
    Wk = np.asarray(Wk, dtype=np.float32)
    bk = np.asarray(bk, dtype=np.float32)
    Wv = np.asarray(Wv, dtype=np.float32)
    bv = np.asarray(bv, dtype=np.float32)
    gamma = np.asarray(gamma, dtype=np.float32)

    shared = {
        "wqT": np.ascontiguousarray(Wq.T),
        "wkT": np.ascontiguousarray(Wk.T),
        "wvT": np.ascontiguousarray(Wv.T),
        "bq": np.ascontiguousarray(bq[:, None]),
        "bk": np.ascontiguousarray(bk[:, None]),
        # softmax rows sum to 1 => v-bias contributes gamma*bv to output
        "bvg": np.ascontiguousarray((gamma.reshape(()) * bv)[:, None]),
        "gam": gamma.reshape(1, 1),
        "ones": np.ones((P, 1), np.float32),
    }
    in_maps = []
    for core in range(NCORES):
        b, h = core // 2, core % 2
        xb = x[b].reshape(C, N)
        xr = np.concatenate(
            [xb[:, h * NQ:(h + 1) * NQ], xb[:, (1 - h) * NQ:(2 - h) * NQ]],
            axis=1)
        m = dict(shared)
        m["x"] = np.ascontiguousarray(xr)
        m["xT"] = np.ascontiguousarray(xr.T)
        in_maps.append(m)
    return in_maps


def assemble_output(results, dtype=np.float32):
    out = np.empty((B, C, N), np.float32)
    for core in range(NCORES):
        b, h = core // 2, core % 2
        out[b][:, h * NQ:(h + 1) * NQ] = results[core]["out"]
    return out.reshape(B, C, HH, WW).astype(dtype, copy=False)


def kernel(x, Wq, bq, Wk, bk, Wv, bv, gamma):
    nc = build_program(repeat=1)
    in_maps = make_in_maps(x, Wq, bq, Wk, bk, Wv, bv, gamma)
    res = run_bass_kernel_spmd(nc, in_maps, list(range(NCORES)))
    return assemble_output(res.results, dtype=np.asarray(x).dtype)


# revision 24
# speedup vs baseline: 480.3545x; 1.0333x over previous
"""Trainium2 Bass kernel for AttentionBlock (B=4, C=256, H=W=64).

Sharding: 8 cores = (batch b, query-half h). Each core holds the full
x[b] (for K over all 4096 key positions) and computes the attention
output for its 2048 query positions. The host permutes x columns so the
core's own query half comes first (key/value order is irrelevant:
softmax and the value contraction sum over all j). The host also
supplies xT (x transposed) so the value contraction needs no on-chip
transposes.

Per-core dataflow (Tile framework, one NeuronCore):
  q = WqT.T @ x[:, :2048] + bq           [32, 2048]
  k = WkT.T @ x + bk                     [32, 4096]
  for each i-superblock (512 queries), software-pipelined with the
  next superblock and with the projections:
    for each j-chunk (128 keys):
      eT[j, i] = k_chunk.T @ q_blk       (PE -> PSUM f32)
      ex = exp(eT)                       (ACT, PSUM->SBUF, f32r)
      z[cin, i]  += xT_chunk.T @ ex      (PE accumulate; reassociated
                                          value path: out = Wv (x attn)
                                          since v = Wv x + bv)
      sums[1, i] += ones.T @ ex          (PE accumulate)
    zs = copy(z)                         (DVE, f32r)
    rg = gamma / sums                    (DVE reciprocal + scale)
    bc = broadcast(rg) to 128 partitions (GPSIMD partition_broadcast)
    out_ps[cout, i] = WvT.T @ zs         (PE)
    out = out_ps * bc + (gamma*bv + x[:, i])   (DVE)
Notes:
 - softmax rows sum to 1, so the v-bias contributes exactly gamma*bv[c]
   to the output; z is computed bias-free and bv folds into the final
   elementwise op.
 - softmax runs without max subtraction: energies are in [-45, 42] for
   this input distribution, well inside f32 exp range.
 - all matmul operands are float32r (full-rate fp32 matmul on TRN2,
   ~tf32 rounding on operand write; measured output error ~3e-4
   relative to an fp64 reference).
"""

import numpy as np

import concourse.bass as bass
import concourse.mybir as mybir
import concourse.tile as tile
from concourse import bacc
from concourse.bass_utils import run_bass_kernel_spmd

AF = mybir.ActivationFunctionType
OP = mybir.AluOpType
F32 = mybir.dt.float32
F32R = mybir.dt.float32r

B, C, HH, WW = 4, 256, 64, 64
N = HH * WW          # 4096 spatial positions
CQ = 32              # q/k channels
NCORES = 8
NQ = N // 2          # 2048 queries per core
P = 128
FB = 512             # free-dim block (one PSUM bank of f32)
JCH = N // P         # 32 j-chunks
ISB = NQ // FB       # 4 i-superblocks
NCH = C // P         # 2 channel chunks
GRP = 4              # j-chunks per energy/exp group


def _emit_body(nc, tc, d):
    """Emit one full forward pass. d: dict of DRAM APs."""
    with (
        tc.tile_pool(name="const", bufs=1) as cpool,
        tc.tile_pool(name="xp", bufs=1) as xpool,
        tc.tile_pool(name="kq", bufs=1) as kqpool,
    ):
        # ---- x: [256, 4096] as 2 partition-chunks; first block DMA'd first
        #      so projections can start ASAP ----
        XBLK = 1024
        x_sb = []
        for cc in range(NCH):
            t = xpool.tile([P, N], F32R, tag=f"x{cc}", name=f"x{cc}")
            x_sb.append(t)
        for cc in range(NCH):
            nc.sync.dma_start(x_sb[cc][:, 0:XBLK], d["x"][cc * P:(cc + 1) * P, 0:XBLK])

        # ---- weights needed by q/k projections ----
        wq_sb, wk_sb, wv_sb, bv_sb = [], [], [], []
        for cc in range(NCH):
            csl = bass.ts(cc, P)
            t = cpool.tile([P, CQ], F32R, tag=f"wq{cc}", name=f"wq{cc}")
            nc.sync.dma_start(t[:], d["wqT"][csl, :])
            wq_sb.append(t)
            t = cpool.tile([P, CQ], F32R, tag=f"wk{cc}", name=f"wk{cc}")
            nc.sync.dma_start(t[:], d["wkT"][csl, :])
            wk_sb.append(t)
        bq_sb = cpool.tile([CQ, 1], F32, tag="bq")
        nc.sync.dma_start(bq_sb[:], d["bq"][:])
        bk_sb = cpool.tile([CQ, 1], F32, tag="bk")
        nc.sync.dma_start(bk_sb[:], d["bk"][:])

        # ---- remaining x blocks ----
        for blk in range(1, N // XBLK):
            sl = bass.ts(blk, XBLK)
            for cc in range(NCH):
                nc.sync.dma_start(x_sb[cc][:, sl], d["x"][cc * P:(cc + 1) * P, sl])

        # ---- remaining constants ----
        for cc in range(NCH):
            csl = bass.ts(cc, P)
            t = cpool.tile([P, C], F32R, tag=f"wv{cc}", name=f"wv{cc}")
            nc.sync.dma_start(t[:], d["wvT"][csl, :])
            wv_sb.append(t)
            t = cpool.tile([P, 1], F32, tag=f"bvg{cc}", name=f"bvg{cc}")
            nc.sync.dma_start(t[:], d["bvg"][csl, :])
            bv_sb.append(t)
        gam_sb = cpool.tile([1, 1], F32, tag="gam")
        nc.sync.dma_start(gam_sb[:], d["gam"][:])
        ones_sb = cpool.tile([P, 1], F32R, tag="ones")
        nc.sync.dma_start(ones_sb[:], d["ones"][:])

        # ---- xT: [4096, 256] -> one tile [128, 32*256]:
        #      partition p, free (a, c) with j = a*128 + p ----
        xt_sb = xpool.tile([P, JCH * C], F32R, tag="xt", name="xt")
        xt_view = d["xT"].rearrange("(a p) c -> p a c", p=P)   # [128, 32, 256]
        for ab in range(2):
            asl = bass.ts(ab, JCH // 2)
            nc.sync.dma_start(
                xt_sb[:, ab * (JCH // 2) * C:(ab + 1) * (JCH // 2) * C],
                xt_view[:, asl, :])

        # ---- q/k projections + attention ----
        # PSUM: ps_e(4 banks) coexists first with ps_proj(4), then with
        # ps_acc(4) after projections close.
        with (
            tc.tile_pool(name="ex", bufs=3) as expool,
            tc.tile_pool(name="ps_e", bufs=1, space="PSUM") as pse,
        ):
            NG = JCH // GRP
            states = []
            q_sb = kqpool.tile([CQ, NQ], F32R, tag="q")
            k_sb = kqpool.tile([CQ, N], F32R, tag="k")

            def emit_eexp(state, g):
                pe_t = pse.tile([P, GRP * FB], F32, tag="pe", name="pe")
                for jj in range(GRP):
                    j = GRP * g + jj
                    nc.tensor.matmul(
                        pe_t[:, bass.ts(jj, FB)],
                        k_sb[:, bass.ts(j, P)],
                        q_sb[:, state["isl"]],
                        start=True, stop=True,
                    )
                ex_t = expool.tile([P, GRP * FB], F32R, tag="ex", name="ex")
                nc.scalar.activation(ex_t[:], pe_t[:], AF.Exp)
                state["exps"][g] = ex_t

            with tc.tile_pool(name="ps_proj", bufs=4, space="PSUM") as psproj:
                def proj(which, nb, pool=None, tag="psp"):
                    w_sb, b_sb, o_sb = ((wq_sb, bq_sb, q_sb) if which == "q"
                                        else (wk_sb, bk_sb, k_sb))
                    ps = (pool or psproj).tile([P, FB], F32, tag=tag,
                                               name="psp")[0:CQ, :]
                    for cc in range(NCH):
                        nc.tensor.matmul(
                            ps[:], w_sb[cc][:], x_sb[cc][:, bass.ts(nb, FB)],
                            start=(cc == 0), stop=(cc == NCH - 1),
                        )
                    nc.vector.tensor_scalar(o_sb[:, bass.ts(nb, FB)], ps[:],
                                            b_sb[:, 0:1], None, op0=OP.add)

                # blk0/blk1 projections upfront; the first energy group is
                # hoisted right after (q0,k0) so its exp overlaps the rest;
                # k4..k7 are deferred into the first superblock's group loop
                # (their x blocks arrive later).
                proj_plan = []
                for blk in range(N // XBLK):
                    for nb in range(blk * XBLK // FB, (blk + 1) * XBLK // FB):
                        if nb < NQ // FB:
                            proj_plan.append(("q", nb))
                        proj_plan.append(("k", nb))
                for which, nb in proj_plan[:6]:
                    proj(which, nb)
                state0 = {"isl": bass.ts(0, FB), "z": None, "sm": None,
                          "exps": {}, "zs": None, "bc": None}
                states.append(state0)
                emit_eexp(state0, 0)
                for which, nb in proj_plan[6:]:
                    proj(which, nb)

            with (
                tc.tile_pool(name="fin", bufs=4) as fpool,
                tc.tile_pool(name="ps_acc", bufs=1, space="PSUM") as psacc,
            ):
                def emit_zg(state, g):
                    if state["z"] is None:
                        state["z"] = [
                            psacc.tile([P, FB], F32, tag=f"z{cc}", name=f"z{cc}")
                            for cc in range(NCH)]
                        state["sm"] = psacc.tile([1, FB], F32, tag="sm", name="sm")
                    ex_t = state["exps"].pop(g)
                    for jj in range(GRP):
                        j = GRP * g + jj
                        exsl = ex_t[:, bass.ts(jj, FB)]
                        for cc in range(NCH):
                            nc.tensor.matmul(
                                state["z"][cc][:],
                                xt_sb[:, j * C + cc * P: j * C + (cc + 1) * P],
                                exsl,
                                start=(j == 0), stop=(j == JCH - 1),
                            )
                        nc.tensor.matmul(
                            state["sm"][:],
                            ones_sb[:, 0:1],
                            exsl,
                            start=(j == 0), stop=(j == JCH - 1),
                        )

                def emit_tail_a(state):
                    state["zs"] = []
                    for cc in range(NCH):
                        t = fpool.tile([P, FB], F32R, tag=f"zs{cc}",
                                       name=f"zs{cc}")
                        nc.vector.tensor_copy(t[:], state["z"][cc][:])
                        state["zs"].append(t)
                    recip_sb = fpool.tile([1, FB], F32, tag="recip",
                                          name="recip")
                    nc.vector.reciprocal(recip_sb[:], state["sm"][:])
                    rg_sb = fpool.tile([1, FB], F32, tag="rg", name="rg")
                    nc.vector.tensor_scalar(rg_sb[:], recip_sb[:],
                                            gam_sb[0:1, 0:1], None, op0=OP.mult)
                    bc_sb = fpool.tile([P, FB], F32, tag="bc_sb", name="bc_sb")
                    nc.gpsimd.partition_broadcast(bc_sb[:], rg_sb[0:1, :])
                    state["bc"] = bc_sb

                def emit_tail_b(state):
                    isl = state["isl"]
                    for co in range(NCH):
                        ops = psacc.tile([P, FB], F32, tag="ops", name="ops")
                        for ci in range(NCH):
                            nc.tensor.matmul(
                                ops[:],
                                wv_sb[ci][:, co * P:(co + 1) * P],
                                state["zs"][ci][:],
                                start=(ci == 0), stop=(ci == NCH - 1),
                            )
                        tmp = fpool.tile([P, FB], F32, tag="tmp", name="tmp")
                        nc.vector.tensor_tensor(tmp[:], ops[:], state["bc"][:],
                                                op=OP.mult)
                        o_sb = fpool.tile([P, FB], F32, tag="osb", name="osb")
                        nc.vector.scalar_tensor_tensor(
                            o_sb[:], tmp[:], bv_sb[co][:, 0:1],
                            x_sb[co][:, isl].bitcast(F32),
                            op0=OP.add, op1=OP.add,
                        )
                        nc.sync.dma_start(d["out"][co * P:(co + 1) * P, isl],
                                          o_sb[:])

                for isb in range(ISB):
                    if isb == 0:
                        state = states[0]
                    else:
                        state = {"isl": bass.ts(isb, FB), "z": None, "sm": None,
                                 "exps": {}, "zs": None, "bc": None}
                        states.append(state)
                    for g in range(NG):
                        if isb == 0 and g == 0:
                            continue  # hoisted into the projection phase
                        emit_eexp(state, g)
                        if isb >= 1:
                            prev = states[isb - 1]
                            if g == 0:
                                emit_zg(prev, NG - 1)
                                emit_tail_a(prev)
                            elif g == 1:
                                emit_tail_b(prev)
                        if g >= 1:
                            emit_zg(state, g - 1)
                last = states[-1]
                emit_zg(last, NG - 1)
                emit_tail_a(last)
                emit_tail_b(last)


_programs = {}


# revision 28
# speedup vs baseline: 480.9982x; 1.0013x over previous
"""Trainium2 Bass kernel for AttentionBlock (B=4, C=256, H=W=64).

Sharding: 8 cores = (batch b, query-half h). Each core holds the full
x[b] (for K over all 4096 key positions) and computes the attention
output for its 2048 query positions. The host permutes x columns so the
core's own query half comes first (key/value order is irrelevant:
softmax and the value contraction sum over all j). The host also
supplies xT (x transposed) so the value contraction needs no on-chip
transposes.

Per-core dataflow (Tile framework, one NeuronCore):
  q = WqT.T @ x[:, :2048] + bq           [32, 2048]
  k = WkT.T @ x + bk                     [32, 4096]
  for each i-superblock (512 queries), software-pipelined with the
  next superblock and with the projections:
    for each j-chunk (128 keys):
      eT[j, i] = k_chunk.T @ q_blk       (PE -> PSUM f32)
      ex = exp(eT)                       (ACT, PSUM->SBUF, f32r)
      z[cin, i]  += xT_chunk.T @ ex      (PE accumulate; reassociated
                                          value path: out = Wv (x attn)
                                          since v = Wv x + bv)
      sums[1, i] += ones.T @ ex          (PE accumulate)
    zs = copy(z)                         (DVE, f32r)
    rg = gamma / sums                    (DVE reciprocal + scale)
    bc = broadcast(rg) to 128 partitions (GPSIMD partition_broadcast)
    out_ps[cout, i] = WvT.T @ zs         (PE)
    out = out_ps * bc + (gamma*bv + x[:, i])   (DVE)
Notes:
 - softmax rows sum to 1, so the v-bias contributes exactly gamma*bv[c]
   to the output; z is computed bias-free and bv folds into the final
   elementwise op.
 - softmax runs without max subtraction: energies are in [-45, 42] for
   this input distribution, well inside f32 exp range.
 - all matmul operands are float32r (full-rate fp32 matmul on TRN2,
   ~tf32 rounding on operand write; measured output error ~3e-4
   relative to an fp64 reference).
"""

import numpy as np

import concourse.bass as bass
import concourse.mybir as mybir
import concourse.tile as tile
from concourse import bacc
from concourse.bass_utils import run_bass_kernel_spmd

AF = mybir.ActivationFunctionType
OP = mybir.AluOpType
F32 = mybir.dt.float32
F32R = mybir.dt.float32r

B, C, HH, WW = 4, 256, 64, 64
N = HH * WW          # 4096 spatial positions
CQ = 32              # q/k channels
NCORES = 8
NQ = N // 2          # 2048 queries per core
P = 128
FB = 512             # free-dim block (one PSUM bank of f32)
JCH = N // P         # 32 j-chunks
ISB = NQ // FB       # 4 i-superblocks
NCH = C // P         # 2 channel chunks
GRP = 4              # j-chunks per energy/exp group


def _emit_body(nc, tc, d):
    """Emit one full forward pass. d: dict of DRAM APs."""
    with (
        tc.tile_pool(name="const", bufs=1) as cpool,
        tc.tile_pool(name="xp", bufs=1) as xpool,
        tc.tile_pool(name="kq", bufs=1) as kqpool,
    ):
        # ---- x: [256, 4096] as 2 partition-chunks; first block DMA'd first
        #      so projections can start ASAP ----
        XBLK = 1024
        x_sb = []
        for cc in range(NCH):
            t = xpool.tile([P, N], F32R, tag=f"x{cc}", name=f"x{cc}")
            x_sb.append(t)
        for cc in range(NCH):
            nc.sync.dma_start(x_sb[cc][:, 0:XBLK], d["x"][cc * P:(cc + 1) * P, 0:XBLK])

        # ---- weights needed by q/k projections ----
        wq_sb, wk_sb, wv_sb, bv_sb = [], [], [], []
        for cc in range(NCH):
            csl = bass.ts(cc, P)
            t = cpool.tile([P, CQ], F32R, tag=f"wq{cc}", name=f"wq{cc}")
            nc.sync.dma_start(t[:], d["wqT"][csl, :])
            wq_sb.append(t)
            t = cpool.tile([P, CQ], F32R, tag=f"wk{cc}", name=f"wk{cc}")
            nc.sync.dma_start(t[:], d["wkT"][csl, :])
            wk_sb.append(t)
        bq_sb = cpool.tile([CQ, 1], F32, tag="bq")
        nc.sync.dma_start(bq_sb[:], d["bq"][:])
        bk_sb = cpool.tile([CQ, 1], F32, tag="bk")
        nc.sync.dma_start(bk_sb[:], d["bk"][:])

        # ---- remaining x blocks ----
        for blk in range(1, N // XBLK):
            sl = bass.ts(blk, XBLK)
            for cc in range(NCH):
                nc.sync.dma_start(x_sb[cc][:, sl], d["x"][cc * P:(cc + 1) * P, sl])

        # ---- remaining constants ----
        for cc in range(NCH):
            csl = bass.ts(cc, P)
            t = cpool.tile([P, C], F32R, tag=f"wv{cc}", name=f"wv{cc}")
            nc.sync.dma_start(t[:], d["wvT"][csl, :])
            wv_sb.append(t)
            t = cpool.tile([P, 1], F32, tag=f"bvg{cc}", name=f"bvg{cc}")
            nc.sync.dma_start(t[:], d["bvg"][csl, :])
            bv_sb.append(t)
        gam_sb = cpool.tile([1, 1], F32, tag="gam")
        nc.sync.dma_start(gam_sb[:], d["gam"][:])
        ones_sb = cpool.tile([P, 1], F32R, tag="ones")
        nc.sync.dma_start(ones_sb[:], d["ones"][:])

        # ---- xT: [4096, 256] -> one tile [128, 32*256]:
        #      partition p, free (a, c) with j = a*128 + p ----
        xt_sb = xpool.tile([P, JCH * C], F32R, tag="xt", name="xt")
        xt_view = d["xT"].rearrange("(a p) c -> p a c", p=P)   # [128, 32, 256]
        for ab in range(2):
            asl = bass.ts(ab, JCH // 2)
            nc.sync.dma_start(
                xt_sb[:, ab * (JCH // 2) * C:(ab + 1) * (JCH // 2) * C],
                xt_view[:, asl, :])

        # ---- q/k projections + attention ----
        # PSUM: ps_e(4 banks) coexists first with ps_proj(4), then with
        # ps_acc(4) after projections close.
        with (
            tc.tile_pool(name="ex", bufs=3) as expool,
            tc.tile_pool(name="ps_e", bufs=1, space="PSUM") as pse,
        ):
            NG = JCH // GRP
            states = []
            q_sb = kqpool.tile([CQ, NQ], F32R, tag="q")
            k_sb = kqpool.tile([CQ, N], F32R, tag="k")

            def emit_eexp(state, g):
                pe_t = pse.tile([P, GRP * FB], F32, tag="pe", name="pe")
                for jj in range(GRP):
                    j = GRP * g + jj
                    nc.tensor.matmul(
                        pe_t[:, bass.ts(jj, FB)],
                        k_sb[:, bass.ts(j, P)],
                        q_sb[:, state["isl"]],
                        start=True, stop=True,
                    )
                ex_t = expool.tile([P, GRP * FB], F32R, tag="ex", name="ex")
                nc.scalar.activation(ex_t[:], pe_t[:], AF.Exp)
                state["exps"][g] = ex_t

            with tc.tile_pool(name="ps_proj", bufs=4, space="PSUM") as psproj:
                def proj(which, nb, pool=None, tag="psp"):
                    w_sb, b_sb, o_sb = ((wq_sb, bq_sb, q_sb) if which == "q"
                                        else (wk_sb, bk_sb, k_sb))
                    ps = (pool or psproj).tile([P, FB], F32, tag=tag,
                                               name="psp")[0:CQ, :]
                    for cc in range(NCH):
                        nc.tensor.matmul(
                            ps[:], w_sb[cc][:], x_sb[cc][:, bass.ts(nb, FB)],
                            start=(cc == 0), stop=(cc == NCH - 1),
                        )
                    nc.vector.tensor_scalar(o_sb[:, bass.ts(nb, FB)], ps[:],
                                            b_sb[:, 0:1], None, op0=OP.add)

                # blk0/blk1 projections upfront; the first energy group is
                # hoisted right after (q0,k0) so its exp overlaps the rest;
                # k4..k7 are deferred into the first superblock's group loop
                # (their x blocks arrive later).
                proj_plan = []
                for blk in range(N // XBLK):
                    for nb in range(blk * XBLK // FB, (blk + 1) * XBLK // FB):
                        if nb < NQ // FB:
                            proj_plan.append(("q", nb))
                        proj_plan.append(("k", nb))
                for which, nb in proj_plan[:6]:
                    proj(which, nb)
                state0 = {"isl": bass.ts(0, FB), "z": None, "sm": None,
                          "exps": {}, "zs": None, "bc": None}
                states.append(state0)
                emit_eexp(state0, 0)
                for which, nb in proj_plan[6:]:
                    proj(which, nb)

            with (
                tc.tile_pool(name="fin", bufs=4) as fpool,
                tc.tile_pool(name="ps_acc", bufs=1, space="PSUM") as psacc,
            ):
                def emit_zg(state, g):
                    if state["z"] is None:
                        state["z"] = [
                            psacc.tile([P, FB], F32, tag=f"z{cc}", name=f"z{cc}")
                            for cc in range(NCH)]
                        state["sm"] = psacc.tile([1, FB], F32, tag="sm", name="sm")
                    ex_t = state["exps"].pop(g)
                    for jj in range(GRP):
                        j = GRP * g + jj
                        exsl = ex_t[:, bass.ts(jj, FB)]
                        nc.tensor.matmul(
                            state["sm"][:],
                            ones_sb[:, 0:1],
                            exsl,
                            start=(j == 0), stop=(j == JCH - 1),
                        )
                        for cc in range(NCH):
                            nc.tensor.matmul(
                                state["z"][cc][:],
                                xt_sb[:, j * C + cc * P: j * C + (cc + 1) * P],
                                exsl,
                                start=(j == 0), stop=(j == JCH - 1),
                            )

                def emit_tail_a(state, last=False):
                    state["zs"] = []
                    for cc in range(NCH):
                        t = fpool.tile([P, FB], F32R, tag=f"zs{cc}",
                                       name=f"zs{cc}")
                        nc.vector.tensor_copy(t[:], state["z"][cc][:])
                        state["zs"].append(t)
                    recip_sb = fpool.tile([1, FB], F32, tag="recip",
                                          name="recip")
                    nc.vector.reciprocal(recip_sb[:], state["sm"][:])
                    rg_sb = fpool.tile([1, FB], F32, tag="rg", name="rg")
                    nc.vector.tensor_scalar(rg_sb[:], recip_sb[:],
                                            gam_sb[0:1, 0:1], None, op0=OP.mult)
                    bc_sb = fpool.tile([P, FB], F32, tag="bc_sb", name="bc_sb")
                    nc.gpsimd.partition_broadcast(bc_sb[:], rg_sb[0:1, :])
                    state["bc"] = bc_sb

                def emit_tail_b(state, last=False):
                    isl = state["isl"]
                    for co in range(NCH):
                        if last and co == 1:
                            ops = pse.tile([P, GRP * FB], F32, tag="pe",
                                           name="opsl")[:, 0:FB]
                        else:
                            ops = psacc.tile([P, FB], F32, tag="ops", name="ops")
                        for ci in range(NCH):
                            nc.tensor.matmul(
                                ops[:],
                                wv_sb[ci][:, co * P:(co + 1) * P],
                                state["zs"][ci][:],
                                start=(ci == 0), stop=(ci == NCH - 1),
                            )
                        tmp = fpool.tile([P, FB], F32, tag="tmp", name="tmp")
                        nc.vector.tensor_tensor(tmp[:], ops[:], state["bc"][:],
                                                op=OP.mult)
                        o_sb = fpool.tile([P, FB], F32, tag="osb", name="osb")
                        nc.vector.scalar_tensor_tensor(
                            o_sb[:], tmp[:], bv_sb[co][:, 0:1],
                            x_sb[co][:, isl].bitcast(F32),
                            op0=OP.add, op1=OP.add,
                        )
                        nc.sync.dma_start(d["out"][co * P:(co + 1) * P, isl],
                                          o_sb[:])

                for isb in range(ISB):
                    if isb == 0:
                        state = states[0]
                    else:
                        state = {"isl": bass.ts(isb, FB), "z": None, "sm": None,
                                 "exps": {}, "zs": None, "bc": None}
                        states.append(state)
                    for g in range(NG):
                        if isb == 0 and g == 0:
                            continue  # hoisted into the projection phase
                        emit_eexp(state, g)
                        if isb >= 1:
                            prev = states[isb - 1]
                            if g == 0:
                                emit_zg(prev, NG - 1)
                                emit_tail_a(prev)
                            elif g == 1:
                                emit_tail_b(prev)
                        if g >= 1:
                            emit_zg(state, g - 1)
                last = states[-1]
                emit_zg(last, NG - 1)
                emit_tail_a(last, last=True)
                emit_tail_b(last, last=True)


_programs = {}


# revision 36
# speedup vs baseline: 521.2788x; 1.0837x over previous
"""Trainium2 Bass kernel for AttentionBlock (B=4, C=256, H=W=64).

Sharding: 8 cores = (batch b, query-half h). Each core holds the full
x[b] (for K over all 4096 key positions) and computes the attention
output for its 2048 query positions. The host permutes x columns so the
core's own query half comes first (key/value order is irrelevant:
softmax and the value contraction sum over all j). The host also
supplies xT (x transposed) so the value contraction needs no on-chip
transposes.

Per-core dataflow (Tile framework, one NeuronCore):
  q = WqT.T @ x[:, :2048] + bq           [32, 2048]
  k = WkT.T @ x + bk                     [32, 4096]
  for each i-superblock (512 queries), software-pipelined with the
  next superblock and with the projections:
    for each j-chunk (128 keys):
      eT[j, i] = k_chunk.T @ q_blk       (PE -> PSUM f32)
      ex = exp(eT)                       (ACT, PSUM->SBUF, f32r)
      z[cin, i]  += xT_chunk.T @ ex      (PE accumulate; reassociated
                                          value path: out = Wv (x attn)
                                          since v = Wv x + bv)
      sums[1, i] += ones.T @ ex          (PE accumulate)
    zs = copy(z)                         (DVE, f32r)
    rg = gamma / sums                    (DVE reciprocal + scale)
    bc = broadcast(rg) to 128 partitions (GPSIMD partition_broadcast)
    out_ps[cout, i] = WvT.T @ zs         (PE)
    out = out_ps * bc + (gamma*bv + x[:, i])   (DVE)
Notes:
 - softmax rows sum to 1, so the v-bias contributes exactly gamma*bv[c]
   to the output; z is computed bias-free and bv folds into the final
   elementwise op.
 - softmax runs without max subtraction: energies are in [-45, 42] for
   this input distribution, well inside f32 exp range.
 - all matmul operands are float32r (full-rate fp32 matmul on TRN2,
   ~tf32 rounding on operand write; measured output error ~3e-4
   relative to an fp64 reference).
"""

import numpy as np

import concourse.bass as bass
import concourse.mybir as mybir
import concourse.tile as tile
from concourse import bacc
from concourse.bass_utils import run_bass_kernel_spmd

AF = mybir.ActivationFunctionType
OP = mybir.AluOpType
F32 = mybir.dt.float32
F32R = mybir.dt.float32r

B, C, HH, WW = 4, 256, 64, 64
N = HH * WW          # 4096 spatial positions
CQ = 32              # q/k channels
NCORES = 8
NQ = N // 2          # 2048 queries per core
P = 128
FB = 512             # free-dim block (one PSUM bank of f32)
JCH = N // P         # 32 j-chunks
ISB = NQ // FB       # 4 i-superblocks
NCH = C // P         # 2 channel chunks
GRP = 4              # j-chunks per energy/exp group


def _emit_body(nc, tc, d):
    """Emit one full forward pass. d: dict of DRAM APs."""
    with (
        tc.tile_pool(name="const", bufs=1) as cpool,
        tc.tile_pool(name="xp", bufs=1) as xpool,
        tc.tile_pool(name="kq", bufs=1) as kqpool,
    ):
        # ---- x: [256, 4096] as 2 partition-chunks; first block DMA'd first
        #      so projections can start ASAP ----
        XBLK = 1024
        x_sb = []
        for cc in range(NCH):
            t = xpool.tile([P, N], F32R, tag=f"x{cc}", name=f"x{cc}")
            x_sb.append(t)
        for cc in range(NCH):
            nc.sync.dma_start(x_sb[cc][:, 0:XBLK], d["x"][cc * P:(cc + 1) * P, 0:XBLK])

        # ---- weights needed by q/k projections ----
        wq_sb, wk_sb, wv_sb, bv_sb = [], [], [], []
        for cc in range(NCH):
            csl = bass.ts(cc, P)
            t = cpool.tile([P, CQ], F32R, tag=f"wq{cc}", name=f"wq{cc}")
            nc.sync.dma_start(t[:], d["wqT"][csl, :])
            wq_sb.append(t)
            t = cpool.tile([P, CQ], F32R, tag=f"wk{cc}", name=f"wk{cc}")
            nc.sync.dma_start(t[:], d["wkT"][csl, :])
            wk_sb.append(t)
        bq_sb = cpool.tile([CQ, 1], F32, tag="bq")
        nc.sync.dma_start(bq_sb[:], d["bq"][:])
        bk_sb = cpool.tile([CQ, 1], F32, tag="bk")
        nc.sync.dma_start(bk_sb[:], d["bk"][:])

        # ---- remaining x blocks ----
        for blk in range(1, N // XBLK):
            sl = bass.ts(blk, XBLK)
            for cc in range(NCH):
                nc.sync.dma_start(x_sb[cc][:, sl], d["x"][cc * P:(cc + 1) * P, sl])

        # ---- remaining constants ----
        for cc in range(NCH):
            csl = bass.ts(cc, P)
            t = cpool.tile([P, C], F32R, tag=f"wv{cc}", name=f"wv{cc}")
            nc.sync.dma_start(t[:], d["wvT"][csl, :])
            wv_sb.append(t)
            t = cpool.tile([P, 1], F32, tag=f"bvg{cc}", name=f"bvg{cc}")
            nc.sync.dma_start(t[:], d["bvg"][csl, :])
            bv_sb.append(t)
        gam_sb = cpool.tile([1, 1], F32, tag="gam")
        nc.sync.dma_start(gam_sb[:], d["gam"][:])
        ones_sb = cpool.tile([P, 1], F32R, tag="ones")
        nc.sync.dma_start(ones_sb[:], d["ones"][:])

        # ---- xT: [4096, 256] -> one tile [128, 32*256]:
        #      partition p, free (a, c) with j = a*128 + p ----
        xt_sb = xpool.tile([P, JCH * C], F32R, tag="xt", name="xt")
        xt_view = d["xT"].rearrange("(a p) c -> p a c", p=P)   # [128, 32, 256]
        for ab in range(2):
            asl = bass.ts(ab, JCH // 2)
            nc.sync.dma_start(
                xt_sb[:, ab * (JCH // 2) * C:(ab + 1) * (JCH // 2) * C],
                xt_view[:, asl, :])

        # ---- q/k projections + attention ----
        # PSUM: ps_e(4 banks) coexists first with ps_proj(4), then with
        # ps_acc(4) after projections close.
        with (
            tc.tile_pool(name="ex", bufs=3) as expool,
            tc.tile_pool(name="ps_e", bufs=1, space="PSUM") as pse,
        ):
            NG = JCH // GRP
            states = []
            q_sb = kqpool.tile([CQ, NQ], F32R, tag="q")
            k_sb = kqpool.tile([CQ, N], F32R, tag="k")

            def emit_eexp(state, g):
                pe_t = pse.tile([P, GRP * FB], F32, tag="pe", name="pe")
                for jj in range(GRP):
                    j = GRP * g + jj
                    nc.tensor.matmul(
                        pe_t[:, bass.ts(jj, FB)],
                        k_sb[:, bass.ts(j, P)],
                        q_sb[:, state["isl"]],
                        start=True, stop=True,
                    )
                ex_t = expool.tile([P, GRP * FB], F32R, tag="ex", name="ex")
                nc.scalar.activation(ex_t[:], pe_t[:], AF.Exp)
                state["exps"][g] = ex_t

            with tc.tile_pool(name="ps_proj", bufs=4, space="PSUM") as psproj:
                def proj(which, nb, pool=None, tag="psp"):
                    w_sb, b_sb, o_sb = ((wq_sb, bq_sb, q_sb) if which == "q"
                                        else (wk_sb, bk_sb, k_sb))
                    ps = (pool or psproj).tile([P, FB], F32, tag=tag,
                                               name="psp")[0:CQ, :]
                    for cc in range(NCH):
                        nc.tensor.matmul(
                            ps[:], w_sb[cc][:], x_sb[cc][:, bass.ts(nb, FB)],
                            start=(cc == 0), stop=(cc == NCH - 1),
                        )
                    nc.vector.tensor_scalar(o_sb[:, bass.ts(nb, FB)], ps[:],
                                            b_sb[:, 0:1], None, op0=OP.add)

                # blk0/blk1 projections upfront; the first energy group is
                # hoisted right after (q0,k0) so its exp overlaps the rest;
                # k4..k7 are deferred into the first superblock's group loop
                # (their x blocks arrive later).
                proj_plan = []
                for blk in range(N // XBLK):
                    for nb in range(blk * XBLK // FB, (blk + 1) * XBLK // FB):
                        if nb < NQ // FB:
                            proj_plan.append(("q", nb))
                        proj_plan.append(("k", nb))
                for which, nb in proj_plan[:6]:
                    proj(which, nb)
                state0 = {"isl": bass.ts(0, FB), "z": None, "sm": None,
                          "exps": {}, "zs": None, "bc": None}
                states.append(state0)
                emit_eexp(state0, 0)
                for which, nb in proj_plan[6:]:
                    proj(which, nb)

            with (
                tc.tile_pool(name="fin", bufs=4) as fpool,
                tc.tile_pool(name="ps_acc", bufs=1, space="PSUM") as psacc,
            ):
                def emit_zg(state, g):
                    if state["z"] is None:
                        state["z"] = [
                            psacc.tile([P, FB], F32, tag=f"z{cc}", name=f"z{cc}")
                            for cc in range(NCH)]
                        state["sm"] = psacc.tile([1, FB], F32, tag="sm", name="sm")
                    ex_t = state["exps"].pop(g)
                    # pre-add exp chunk pairs on DVE (idle capacity), halving
                    # the ones-matmul count on the PE critical path; the
                    # ones-contraction over a pair-sum is mathematically the
                    # same sum over both chunks
                    pairs = []
                    for pp in range(GRP // 2):
                        pt = fpool.tile([P, FB], F32R, tag=f"smp{pp}",
                                        name=f"smp{pp}")
                        nc.vector.tensor_tensor(
                            pt[:], ex_t[:, bass.ts(2 * pp, FB)],
                            ex_t[:, bass.ts(2 * pp + 1, FB)], op=OP.add)
                        pairs.append(pt)
                    for pp, pt in enumerate(pairs):
                        nc.tensor.matmul(
                            state["sm"][:],
                            ones_sb[:, 0:1],
                            pt[:],
                            start=(g == 0 and pp == 0),
                            stop=(g == NG - 1 and pp == GRP // 2 - 1),
                        )
                    for jj in range(GRP):
                        j = GRP * g + jj
                        exsl = ex_t[:, bass.ts(jj, FB)]
                        for cc in range(NCH):
                            nc.tensor.matmul(
                                state["z"][cc][:],
                                xt_sb[:, j * C + cc * P: j * C + (cc + 1) * P],
                                exsl,
                                start=(j == 0), stop=(j == JCH - 1),
                            )

                def emit_tail_a(state, last=False):
                    state["zs"] = []
                    for cc in range(NCH):
                        t = fpool.tile([P, FB], F32R, tag=f"zs{cc}",
                                       name=f"zs{cc}")
                        nc.vector.tensor_copy(t[:], state["z"][cc][:])
                        state["zs"].append(t)
                    recip_sb = fpool.tile([1, FB], F32, tag="recip",
                                          name="recip")
                    nc.vector.reciprocal(recip_sb[:], state["sm"][:])
                    rg_sb = fpool.tile([1, FB], F32, tag="rg", name="rg")
                    nc.vector.tensor_scalar(rg_sb[:], recip_sb[:],
                                            gam_sb[0:1, 0:1], None, op0=OP.mult)
                    bc_sb = fpool.tile([P, FB], F32, tag="bc_sb", name="bc_sb")
                    nc.gpsimd.partition_broadcast(bc_sb[:], rg_sb[0:1, :])
                    state["bc"] = bc_sb

                def emit_tail_b(state, last=False):
                    isl = state["isl"]
                    for co in range(NCH):
                        if last and co == 1:
                            ops = pse.tile([P, GRP * FB], F32, tag="pe",
                                           name="opsl")[:, 0:FB]
                        else:
                            ops = psacc.tile([P, FB], F32, tag="ops", name="ops")
                        for ci in range(NCH):
                            nc.tensor.matmul(
                                ops[:],
                                wv_sb[ci][:, co * P:(co + 1) * P],
                                state["zs"][ci][:],
                                start=(ci == 0), stop=(ci == NCH - 1),
                            )
                        tmp = fpool.tile([P, FB], F32, tag="tmp", name="tmp")
                        nc.vector.tensor_tensor(tmp[:], ops[:], state["bc"][:],
                                                op=OP.mult)
                        o_sb = fpool.tile([P, FB], F32, tag="osb", name="osb")
                        nc.vector.scalar_tensor_tensor(
                            o_sb[:], tmp[:], bv_sb[co][:, 0:1],
                            x_sb[co][:, isl].bitcast(F32),
                            op0=OP.add, op1=OP.add,
                        )
                        nc.sync.dma_start(d["out"][co * P:(co + 1) * P, isl],
                                          o_sb[:])

                for isb in range(ISB):
                    if isb == 0:
                        state = states[0]
                    else:
                        state = {"isl": bass.ts(isb, FB), "z": None, "sm": None,
                                 "exps": {}, "zs": None, "bc": None}
                        states.append(state)
                    for g in range(NG):
                        if isb == 0 and g == 0:
                            continue  # hoisted into the projection phase
                        emit_eexp(state, g)
                        if isb >= 1:
                            prev = states[isb - 1]
                            if g == 0:
                                emit_zg(prev, NG - 1)
                                emit_tail_a(prev)
                            elif g == 1:
                                emit_tail_b(prev)
                        if g >= 1:
                            emit_zg(state, g - 1)
                last = states[-1]
                emit_zg(last, NG - 1)
                emit_tail_a(last, last=True)
                emit_tail_b(last, last=True)


_programs = {}


# revision 38
# speedup vs baseline: 524.1118x; 1.0054x over previous
"""Trainium2 Bass kernel for AttentionBlock (B=4, C=256, H=W=64).

Sharding: 8 cores = (batch b, query-half h). Each core holds the full
x[b] (for K over all 4096 key positions) and computes the attention
output for its 2048 query positions. The host permutes x columns so the
core's own query half comes first (key/value order is irrelevant:
softmax and the value contraction sum over all j). The host also
supplies xT (x transposed) so the value contraction needs no on-chip
transposes.

Per-core dataflow (Tile framework, one NeuronCore):
  q = WqT.T @ x[:, :2048] + bq           [32, 2048]
  k = WkT.T @ x + bk                     [32, 4096]
  for each i-superblock (512 queries), software-pipelined with the
  next superblock and with the projections:
    for each j-chunk (128 keys):
      eT[j, i] = k_chunk.T @ q_blk       (PE -> PSUM f32)
      ex = exp(eT)                       (ACT, PSUM->SBUF, f32r)
      z[cin, i]  += xT_chunk.T @ ex      (PE accumulate; reassociated
                                          value path: out = Wv (x attn)
                                          since v = Wv x + bv)
      sums[1, i] += ones.T @ (ex_a + ex_b)  (PE accumulate over exp chunk
                                          PAIRS pre-added on the DVE --
                                          halves the ones-matmul columns
                                          on the PE critical path)
    zs = copy(z)                         (DVE, f32r)
    rg = gamma / sums                    (DVE reciprocal + scale)
    bc = broadcast(rg) to 128 partitions (GPSIMD partition_broadcast)
    out_ps[cout, i] = WvT.T @ zs         (PE)
    out = out_ps * bc + (gamma*bv + x[:, i])   (DVE)
Notes:
 - softmax rows sum to 1, so the v-bias contributes exactly gamma*bv[c]
   to the output; z is computed bias-free and bv folds into the final
   elementwise op.
 - softmax runs without max subtraction: energies are in [-45, 42] for
   this input distribution, well inside f32 exp range.
 - all matmul operands are float32r (full-rate fp32 matmul on TRN2,
   ~tf32 rounding on operand write; measured output error ~3e-4
   relative to an fp64 reference).
"""

import numpy as np

import concourse.bass as bass
import concourse.mybir as mybir
import concourse.tile as tile
from concourse import bacc
from concourse.bass_utils import run_bass_kernel_spmd

AF = mybir.ActivationFunctionType
OP = mybir.AluOpType
F32 = mybir.dt.float32
F32R = mybir.dt.float32r

B, C, HH, WW = 4, 256, 64, 64
N = HH * WW          # 4096 spatial positions
CQ = 32              # q/k channels
NCORES = 8
NQ = N // 2          # 2048 queries per core
P = 128
FB = 512             # free-dim block (one PSUM bank of f32)
JCH = N // P         # 32 j-chunks
ISB = NQ // FB       # 4 i-superblocks
NCH = C // P         # 2 channel chunks
GRP = 4              # j-chunks per energy/exp group


def _emit_body(nc, tc, d):
    """Emit one full forward pass. d: dict of DRAM APs."""
    with (
        tc.tile_pool(name="const", bufs=1) as cpool,
        tc.tile_pool(name="xp", bufs=1) as xpool,
        tc.tile_pool(name="kq", bufs=1) as kqpool,
    ):
        # ---- x: [256, 4096] as 2 partition-chunks; first block DMA'd first
        #      so projections can start ASAP ----
        XBLK = 1024
        x_sb = []
        for cc in range(NCH):
            t = xpool.tile([P, N], F32R, tag=f"x{cc}", name=f"x{cc}")
            x_sb.append(t)
        for cc in range(NCH):
            nc.sync.dma_start(x_sb[cc][:, 0:XBLK], d["x"][cc * P:(cc + 1) * P, 0:XBLK])

        # ---- weights needed by q/k projections ----
        wq_sb, wk_sb, wv_sb, bv_sb = [], [], [], []
        for cc in range(NCH):
            csl = bass.ts(cc, P)
            t = cpool.tile([P, CQ], F32R, tag=f"wq{cc}", name=f"wq{cc}")
            nc.sync.dma_start(t[:], d["wqT"][csl, :])
            wq_sb.append(t)
            t = cpool.tile([P, CQ], F32R, tag=f"wk{cc}", name=f"wk{cc}")
            nc.sync.dma_start(t[:], d["wkT"][csl, :])
            wk_sb.append(t)
        bq_sb = cpool.tile([CQ, 1], F32, tag="bq")
        nc.sync.dma_start(bq_sb[:], d["bq"][:])
        bk_sb = cpool.tile([CQ, 1], F32, tag="bk")
        nc.sync.dma_start(bk_sb[:], d["bk"][:])

        # ---- remaining x blocks and xT quarters, interleaved so each
        #      arrives just before its consumers (late k-projections and
        #      the z-contraction groups of the first superblock) ----
        for blk in (1, 2):
            sl = bass.ts(blk, XBLK)
            for cc in range(NCH):
                nc.sync.dma_start(x_sb[cc][:, sl], d["x"][cc * P:(cc + 1) * P, sl])

        xt_sb = xpool.tile([P, JCH * C], F32R, tag="xt", name="xt")
        xt_view = d["xT"].rearrange("(a p) c -> p a c", p=P)   # [128, 32, 256]

        def dma_xtq(ab):
            asl = bass.ts(ab, JCH // 4)
            nc.sync.dma_start(
                xt_sb[:, ab * (JCH // 4) * C:(ab + 1) * (JCH // 4) * C],
                xt_view[:, asl, :])

        dma_xtq(0)
        sl = bass.ts(3, XBLK)
        for cc in range(NCH):
            nc.sync.dma_start(x_sb[cc][:, sl], d["x"][cc * P:(cc + 1) * P, sl])
        dma_xtq(1)
        dma_xtq(2)
        dma_xtq(3)

        # ---- remaining constants ----
        for cc in range(NCH):
            csl = bass.ts(cc, P)
            t = cpool.tile([P, C], F32R, tag=f"wv{cc}", name=f"wv{cc}")
            nc.sync.dma_start(t[:], d["wvT"][csl, :])
            wv_sb.append(t)
            t = cpool.tile([P, 1], F32, tag=f"bvg{cc}", name=f"bvg{cc}")
            nc.sync.dma_start(t[:], d["bvg"][csl, :])
            bv_sb.append(t)
        gam_sb = cpool.tile([1, 1], F32, tag="gam")
        nc.sync.dma_start(gam_sb[:], d["gam"][:])
        ones_sb = cpool.tile([P, 1], F32R, tag="ones")
        nc.sync.dma_start(ones_sb[:], d["ones"][:])

        # ---- q/k projections + attention ----
        # PSUM: ps_e(4 banks) coexists first with ps_proj(4), then with
        # ps_acc(4) after projections close.
        with (
            tc.tile_pool(name="ex", bufs=4) as expool,
            tc.tile_pool(name="ps_e", bufs=1, space="PSUM") as pse,
        ):
            NG = JCH // GRP
            states = []
            q_sb = kqpool.tile([CQ, NQ], F32R, tag="q")
            k_sb = kqpool.tile([CQ, N], F32R, tag="k")

            def emit_eexp(state, g):
                pe_t = pse.tile([P, GRP * FB], F32, tag="pe", name="pe")
                for jj in range(GRP):
                    j = GRP * g + jj
                    nc.tensor.matmul(
                        pe_t[:, bass.ts(jj, FB)],
                        k_sb[:, bass.ts(j, P)],
                        q_sb[:, state["isl"]],
                        start=True, stop=True,
                    )
                ex_t = expool.tile([P, GRP * FB], F32R, tag="ex", name="ex")
                nc.scalar.activation(ex_t[:], pe_t[:], AF.Exp)
                state["exps"][g] = ex_t

            with tc.tile_pool(name="ps_proj", bufs=4, space="PSUM") as psproj:
                def proj(which, nb, pool=None, tag="psp"):
                    w_sb, b_sb, o_sb = ((wq_sb, bq_sb, q_sb) if which == "q"
                                        else (wk_sb, bk_sb, k_sb))
                    ps = (pool or psproj).tile([P, FB], F32, tag=tag,
                                               name="psp")[0:CQ, :]
                    for cc in range(NCH):
                        nc.tensor.matmul(
                            ps[:], w_sb[cc][:], x_sb[cc][:, bass.ts(nb, FB)],
                            start=(cc == 0), stop=(cc == NCH - 1),
                        )
                    nc.vector.tensor_scalar(o_sb[:, bass.ts(nb, FB)], ps[:],
                                            b_sb[:, 0:1], None, op0=OP.add)

                # blk0/blk1 projections upfront; the first energy group is
                # hoisted right after (q0,k0) so its exp overlaps the rest;
                # k4..k7 are deferred into the first superblock's group loop
                # (their x blocks arrive later).
                proj_plan = [("q", 0), ("k", 0), ("q", 1), ("k", 1),
                             ("q", 2), ("k", 2), ("q", 3), ("k", 3)]
                for which, nb in proj_plan[:6]:
                    proj(which, nb)
                state0 = {"isl": bass.ts(0, FB), "z": None, "sm": None,
                          "exps": {}, "zs": None, "bc": None}
                states.append(state0)
                emit_eexp(state0, 0)
                for which, nb in proj_plan[6:]:
                    proj(which, nb)
                state0["late_k"] = [4, 5, 6, 7]

            with (
                tc.tile_pool(name="fin", bufs=4) as fpool,
                tc.tile_pool(name="ps_acc", bufs=1, space="PSUM") as psacc,
            ):
                def emit_zg(state, g):
                    if state["z"] is None:
                        state["z"] = [
                            psacc.tile([P, FB], F32, tag=f"z{cc}", name=f"z{cc}")
                            for cc in range(NCH)]
                        state["sm"] = psacc.tile([1, FB], F32, tag="sm", name="sm")
                    ex_t = state["exps"].pop(g)
                    # pre-add exp chunk pairs on DVE (idle capacity), halving
                    # the ones-matmul count on the PE critical path; the
                    # ones-contraction over a pair-sum is mathematically the
                    # same sum over both chunks
                    pairs = []
                    for pp in range(GRP // 2):
                        pt = fpool.tile([P, FB], F32R, tag=f"smp{pp}",
                                        name=f"smp{pp}")
                        nc.vector.tensor_tensor(
                            pt[:], ex_t[:, bass.ts(2 * pp, FB)],
                            ex_t[:, bass.ts(2 * pp + 1, FB)], op=OP.add)
                        pairs.append(pt)
                    for pp, pt in enumerate(pairs):
                        nc.tensor.matmul(
                            state["sm"][:],
                            ones_sb[:, 0:1],
                            pt[:],
                            start=(g == 0 and pp == 0),
                            stop=(g == NG - 1 and pp == GRP // 2 - 1),
                        )
                    for jj in range(GRP):
                        j = GRP * g + jj
                        exsl = ex_t[:, bass.ts(jj, FB)]
                        for cc in range(NCH):
                            nc.tensor.matmul(
                                state["z"][cc][:],
                                xt_sb[:, j * C + cc * P: j * C + (cc + 1) * P],
                                exsl,
                                start=(j == 0), stop=(j == JCH - 1),
                            )

                def emit_tail_a(state, last=False):
                    state["zs"] = []
                    for cc in range(NCH):
                        t = fpool.tile([P, FB], F32R, tag=f"zs{cc}",
                                       name=f"zs{cc}")
                        nc.vector.tensor_copy(t[:], state["z"][cc][:])
                        state["zs"].append(t)
                    recip_sb = fpool.tile([1, FB], F32, tag="recip",
                                          name="recip")
                    nc.vector.reciprocal(recip_sb[:], state["sm"][:])
                    rg_sb = fpool.tile([1, FB], F32, tag="rg", name="rg")
                    nc.vector.tensor_scalar(rg_sb[:], recip_sb[:],
                                            gam_sb[0:1, 0:1], None, op0=OP.mult)
                    bc_sb = fpool.tile([P, FB], F32, tag="bc_sb", name="bc_sb")
                    nc.gpsimd.partition_broadcast(bc_sb[:], rg_sb[0:1, :])
                    state["bc"] = bc_sb

                def emit_tail_b(state, last=False):
                    isl = state["isl"]
                    for co in range(NCH):
                        if last and co == 1:
                            ops = pse.tile([P, GRP * FB], F32, tag="pe",
                                           name="opsl")[:, 0:FB]
                        else:
                            ops = psacc.tile([P, FB], F32, tag="ops", name="ops")
                        for ci in range(NCH):
                            nc.tensor.matmul(
                                ops[:],
                                wv_sb[ci][:, co * P:(co + 1) * P],
                                state["zs"][ci][:],
                                start=(ci == 0), stop=(ci == NCH - 1),
                            )
                        tmp = fpool.tile([P, FB], F32, tag="tmp", name="tmp")
                        nc.vector.tensor_tensor(tmp[:], ops[:], state["bc"][:],
                                                op=OP.mult)
                        o_sb = fpool.tile([P, FB], F32, tag="osb", name="osb")
                        nc.vector.scalar_tensor_tensor(
                            o_sb[:], tmp[:], bv_sb[co][:, 0:1],
                            x_sb[co][:, isl].bitcast(F32),
                            op0=OP.add, op1=OP.add,
                        )
                        nc.sync.dma_start(d["out"][co * P:(co + 1) * P, isl],
                                          o_sb[:])

                for isb in range(ISB):
                    if isb == 0:
                        state = states[0]
                    else:
                        state = {"isl": bass.ts(isb, FB), "z": None, "sm": None,
                                 "exps": {}, "zs": None, "bc": None}
                        states.append(state)
                    zlag = 2 if isb == 0 else 1
                    for g in range(NG):
                        if isb == 0 and g == 0:
                            continue  # hoisted into the projection phase
                        if isb == 0 and state.get("late_k"):
                            proj("k", state["late_k"].pop(0),
                                 pool=psacc, tag="ops")
                        emit_eexp(state, g)
                        if isb >= 1:
                            prev = states[isb - 1]
                            if g == 0:
                                for pg in range(NG - (2 if prev.get("lag2")
                                                      else 1), NG):
                                    emit_zg(prev, pg)
                                emit_tail_a(prev)
                            elif g == 1:
                                emit_tail_b(prev)
                        if g >= zlag:
                            emit_zg(state, g - zlag)
                    state["lag2"] = (zlag == 2)
                last = states[-1]
                for pg in range(NG - (2 if last.get("lag2") else 1), NG):
                    emit_zg(last, pg)
                emit_tail_a(last, last=True)
                emit_tail_b(last, last=True)


_programs = {}
